# revision 73
# baseline (speedup 1.0000x reference)
"""DurationConditioningProjector Trainium2 kernel.

Data-parallel over batch B=16 across 8 NeuronCores (2 items per core).
Host does input relayout only (transpose/cumsum/bias folding); all model
compute runs on-device. The emitted program is specialized to the union
(over cores) of phoneme-chunk/frame-slab intersections derived from the
host-computed cumsum; per-core band masks built on-device from that
core's own cum make the union slack contribute exactly zero, so one
SPMD program serves all cores and stays correct for any input (new
inputs recompile via the cache key).

Per-item layout: residual x as (C=128 partitions, T free) fp32 in SBUF.
- Length-regulator upsample AS MATMUL: per phoneme chunk g a band mask
  D[m,t] = [t>=cum[m-1]] - [t>=cum[m]] is built once over the chunk's
  frame band (2 is_ge + 1 sub against a global iota); each slab's PSUM
  gets one sliced matmul per intersecting chunk, plus the sin/cos
  pos-emb matmul and an in_b+pos_b rank-1.
- LayerNorm stats as (8,F) PSUM rows via one-hot-column matmuls; row
  math on (8,F) tiles; per-frame scale/offset broadcast to (128,F) by
  0-stride DMA from a DRAM bounce; normalize = 2 DVE TTs + fused ACT
  gelu(g*z+b).
- 3 dilated causal conv layers: 31 shifted bf16 matmuls per slab into
  PSUM; residual add fused in one scalar_tensor_tensor. Slab-level
  software pipelining: item 1's upsample and each step's stats are
  injected into the running conv stream so the PE never drains.
"""
import sys
sys.path.insert(0, '/opt/trn_rl_repo')

import math
import os
import numpy as np

import concourse.bass as bass
import concourse.mybir as mybir
import concourse.tile as tile
from concourse import bacc
from concourse import bass_utils

dt = mybir.dt
Alu = mybir.AluOpType
ActF = mybir.ActivationFunctionType
_GELU = ActF.Tanh if os.environ.get('KSIM_TANH') else ActF.Gelu

B, N, DIN, C, DOUT, K, L = 16, 1024, 256, 128, 256, 31, 3
NCORES = 8
BPC = B // NCORES
TWO_PI = 2.0 * math.pi
EPS = 1e-5


def _ceil_to(x, m):
    return (x + m - 1) // m * m


def build_nc(T, chunk_bands, slab_bands):
    # chunk_bands[b] = tuple of (g, lo, hi) mask pieces (width <= 512)
    # slab_bands[b][si] = tuple of (piece_idx, a0, a1): piece contributes
    # to absolute frames [a0, a1) within slab si.
    TP = _ceil_to(T, 128)
    NT = TP // 128
    F = TP // 8
    assert F % 16 == 0 and F <= 512
    NCH = N // 128
    WG = 32 * ((max((hi - lo) for cb in chunk_bands
                    for (_, lo, hi) in cb) + 31) // 32)
    NPIECE = sum(len(cb) for cb in chunk_bands)

    nc = bacc.Bacc("TRN2", target_bir_lowering=False, debug=False)

    pooledT = nc.dram_tensor("pooledT", [BPC, 128, 2, N], dt.float32, kind="ExternalInput").ap()
    cumd = nc.dram_tensor("cumd", [BPC, N + 1], dt.float32, kind="ExternalInput").ap()
    rel_pos = nc.dram_tensor("rel_pos", [BPC, T], dt.float32, kind="ExternalInput").ap()
    in_wT = nc.dram_tensor("in_wT", [DIN, C], dt.float32, kind="ExternalInput").ap()
    pos_wT = nc.dram_tensor("pos_wT", [C, C], dt.float32, kind="ExternalInput").ap()
    conv_wr = nc.dram_tensor("conv_wr", [L, K, C, C], dt.float32, kind="ExternalInput").ap()
    conv_b = nc.dram_tensor("conv_b", [L, C], dt.float32, kind="ExternalInput").ap()
    ln_g = nc.dram_tensor("ln_g", [L, C], dt.float32, kind="ExternalInput").ap()
    ln_b = nc.dram_tensor("ln_b", [L, C], dt.float32, kind="ExternalInput").ap()
    out_ln_g = nc.dram_tensor("out_ln_g", [C], dt.float32, kind="ExternalInput").ap()
    out_ln_b = nc.dram_tensor("out_ln_b", [C], dt.float32, kind="ExternalInput").ap()
    out_wT = nc.dram_tensor("out_wT", [C, DOUT], dt.float32, kind="ExternalInput").ap()
    out_b = nc.dram_tensor("out_b", [DOUT], dt.float32, kind="ExternalInput").ap()
    ipb_row = nc.dram_tensor("ipb_row", [1, C], dt.float32, kind="ExternalInput").ap()
    out = nc.dram_tensor("out", [BPC, T, DOUT], dt.bfloat16, kind="ExternalOutput").ap()

    iota_c = nc.inline_tensor(
        np.broadcast_to(np.arange(512, dtype=np.float32), (128, 512)).copy(), "iotac")
    oh = np.zeros((8, 8), np.float32)
    np.fill_diagonal(oh, 1.0)
    onehot_c = nc.inline_tensor(
        np.broadcast_to(oh[None, :, :], (128, 8, 8)).copy(), "onehotc")
    ohdr = np.zeros((2, 2, 16), np.float32)
    for q_ in range(2):
        ohdr[q_, 0, 2 * q_] = 1.0
        ohdr[q_, 1, 2 * q_ + 1] = 1.0
    ohdr_c = nc.inline_tensor(
        np.broadcast_to(ohdr[None], (128, 2, 2, 16)).copy(), "ohdrc")
    half = C // 2
    freqs = np.exp(-math.log(10000.0) * np.arange(half, dtype=np.float64) / (half - 1))
    fc2pi_c = nc.inline_tensor(
        (np.concatenate([freqs, freqs]) / TWO_PI).astype(np.float32)[:, None], "fc2pic")
    coff_np = np.zeros((C, 1), np.float32)
    coff_np[half:] = 0.25
    coff_c = nc.inline_tensor(coff_np, "coffc")

    dils = [2 ** i for i in range(L)]
    HPAD = (K - 1) * dils[-1]

    with tile.TileContext(nc) as tc:
        import contextlib
        ctx = contextlib.ExitStack()
        with ctx:
            pers = ctx.enter_context(tc.tile_pool(name="pers", bufs=1))
            pp = ctx.enter_context(tc.tile_pool(name="pp", bufs=2))
            psum = ctx.enter_context(tc.tile_pool(name="ps", bufs=1, space="PSUM"))
            dram = ctx.enter_context(tc.tile_pool(name="dr", bufs=1, space="DRAM"))

            def ptile(shape, d, nm, bufs=1, pool=None):
                pool = pool or pers
                return pool.tile(shape, d, tag=nm, name=nm, bufs=bufs)

            # ---------- shared setup ----------
            iot512 = ptile([128, 512], dt.float32, "iot512")
            nc.sync.dma_start(out=iot512, in_=iota_c.ap())
            fc2pi = ptile([C, 1], dt.float32, "fc2pi")
            nc.scalar.dma_start(out=fc2pi, in_=fc2pi_c.ap())
            coff = ptile([C, 1], dt.float32, "coff")
            nc.scalar.dma_start(out=coff, in_=coff_c.ap())
            onesF = ptile([1, F], dt.bfloat16, "onesF")
            nc.vector.memset(onesF, 1.0)
            ohdr_f = ptile([128, 2, 2, 16], dt.float32, "ohdrf", pool=pp)
            nc.scalar.dma_start(out=ohdr_f, in_=ohdr_c.ap())
            ohdr8 = ptile([128, 2, 2, 16], dt.float8e4, "ohdr8")
            nc.vector.tensor_copy(ohdr8, ohdr_f)
            zcol = ptile([128, 1], dt.float32, "zcol")
            nc.vector.memset(zcol, 0.0)

            def col(src_1d, nm):
                t = ptile([C, 1], dt.float32, nm)
                nc.scalar.dma_start(out=t, in_=src_1d[:, None])
                return t

            ipb_col = ptile([C, 1], dt.float32, "ipbcol")
            nc.scalar.dma_start(out=ipb_col, in_=bass.AP(
                tensor=ipb_row.tensor, offset=ipb_row.offset, ap=[[1, C], [C, 1]]))
            conv_b_col = [col(conv_b[l], f"cb{l}") for l in range(L)]
            g_col = [col(ln_g[l], f"g{l}") for l in range(L)]
            b_col = [col(ln_b[l], f"b{l}") for l in range(L)]
            og_col = col(out_ln_g, "og")
            ob_col = col(out_ln_b, "ob")

            outbB = ptile([128, DOUT], dt.float32, "outbB")
            nc.scalar.dma_start(out=outbB, in_=bass.AP(
                tensor=out_b.tensor, offset=out_b.offset, ap=[[0, 128], [1, DOUT]]))

            inw_f = ptile([128, 2, C], dt.float32, "inwf")
            nc.scalar.dma_start(out=inw_f, in_=in_wT.rearrange("(a p) c -> p a c", p=128))
            inw_bf = ptile([128, 2, C], dt.bfloat16, "inwbf")
            nc.vector.tensor_copy(inw_bf, inw_f)
            posw_f = ptile([C, C], dt.float32, "poswf")
            nc.scalar.dma_start(out=posw_f, in_=pos_wT)
            posw_bf = ptile([C, C], dt.bfloat16, "poswbf")
            nc.vector.tensor_copy(posw_bf, posw_f)
            outw_f = ptile([C, DOUT], dt.float32, "outwf")
            nc.scalar.dma_start(out=outw_f, in_=out_wT)
            outw_bf = ptile([C, DOUT], dt.bfloat16, "outwbf")
            cw_bf = [ptile([128, K, C], dt.bfloat16, f"cw{l}")
                     for l in range(L)]

            def emit_deferred_weight_casts():
                # conv-weight staging + bf16 conversions queued only after
                # the upsample chain's ops, so the startup critical path
                # (DVE in-order queue) is not delayed
                nc.vector.tensor_copy(outw_bf, outw_f)
                for l in range(L):
                    cwf = ptile([128, K, C], dt.float32, "cwstage", pool=pp,
                                bufs=1)
                    (nc.scalar, nc.sync, nc.gpsimd)[l].dma_start(
                        out=cwf, in_=conv_wr[l].rearrange("k p c -> p k c"))
                    nc.vector.tensor_copy(cw_bf[l], cwf)

            # ---------- per-item persistent ----------
            xs, hs = [], []
            for b in range(BPC):
                xs.append(ptile([C, TP], dt.float32, f"x{b}"))
                h = ptile([C, HPAD + TP], dt.bfloat16, f"h{b}")
                nc.vector.memset(h[:, 0:HPAD], 0.0)
                hs.append(h)
            ybfs = [ptile([C, TP], dt.bfloat16, f"ybf{b}") for b in range(BPC)]

            # ---------- cum band edges (host-computed cumd) ----------
            cumws, cumw0s = [], []
            def emit_cum(b):
                cumw = ptile([128, NCH], dt.float32, f"cumw{b}", pool=pp)
                nc.sync.dma_start(out=cumw, in_=bass.AP(
                    tensor=cumd.tensor, offset=cumd.offset + b * (N + 1) + 1,
                    ap=[[1, 128], [128, NCH]]))
                cumw0 = ptile([128, NCH], dt.float32, f"cumw0{b}", pool=pp)
                nc.sync.dma_start(out=cumw0, in_=bass.AP(
                    tensor=cumd.tensor, offset=cumd.offset + b * (N + 1),
                    ap=[[1, 128], [128, NCH]]))
                cumws.append(cumw)
                cumw0s.append(cumw0)

            # ---------- per-chunk band masks ----------
            # D[m, t] = [t >= cum[m-1]] - [t >= cum[m]] over the chunk's
            # band; all masks depend only on iotG + cum so they pipeline
            # far ahead of the PE.
            Dg = {}
            def ensure_mask(b, pi):
                # lazily build D = [t>=cum0]*[t<cum1] (2 DVE ops) so mask
                # work interleaves with the consuming slab instead of
                # front-loading the vector queue
                if (b, pi) in Dg:
                    return Dg[(b, pi)]
                g, lo, hi = chunk_bands[b][pi]
                w = hi - lo
                bv0p = pp.tile([128, 1], dt.float32, tag="bvp", name="bvp",
                               bufs=4)
                nc.vector.tensor_scalar(bv0p, cumw0s[b][:, g:g + 1], float(lo),
                                        None, Alu.subtract)
                bv1p = pp.tile([128, 1], dt.float32, tag="bvp", name="bvp",
                               bufs=4)
                nc.vector.tensor_scalar(bv1p, cumws[b][:, g:g + 1], float(lo),
                                        None, Alu.subtract)
                S0 = pp.tile([128, WG], dt.bfloat16, tag="S0", name="S0", bufs=2)
                nc.vector.tensor_scalar(S0[:, 0:w], iot512[:, 0:w],
                                        bv0p, None, Alu.is_ge)
                D = pp.tile([128, WG], dt.bfloat16, tag="Dg", name="Dg",
                            bufs=NPIECE)
                nc.vector.scalar_tensor_tensor(D[:, 0:w], iot512[:, 0:w],
                                               bv1p, S0[:, 0:w],
                                               Alu.is_lt, Alu.mult)
                Dg[(b, pi)] = D
                return D

            # ---------- P'' projection (SBUF-resident, bf16) ----------
            psts = []
            def emit_proj(b):
                ptf = pp.tile([128, 2, N], dt.float32, tag="ptf", name="ptf", bufs=1)
                src = pooledT[b]
                ptb = ptile([128, 2, N], dt.bfloat16, f"ptb{b}", pool=pp, bufs=1)
                for qi, qe in enumerate((nc.sync, nc.scalar, nc.gpsimd, nc.sync)):
                    qe.dma_start(out=ptf[:, :, qi * 256:(qi + 1) * 256],
                                 in_=src[:, :, qi * 256:(qi + 1) * 256])
                    nc.vector.tensor_copy(ptb[:, :, qi * 256:(qi + 1) * 256],
                                          ptf[:, :, qi * 256:(qi + 1) * 256])
                pst = ptile([128, NCH, C], dt.bfloat16, f"pst{b}")
                for nchunk in range(NCH):
                    ppsf = psum.tile([128, F], dt.float32, tag="xps", name="xps", bufs=2)
                    pps = ppsf[:, 0:C]
                    for dh in range(2):
                        nc.tensor.matmul(pps, ptb[:, dh, 128 * nchunk:128 * (nchunk + 1)],
                                         inw_bf[:, dh, :], start=(dh == 0), stop=(dh == 1))
                    nc.vector.tensor_copy(pst[:, nchunk, :], pps)
                psts.append(pst)

            # ---------- upsample + posemb, one PSUM accumulation per slab ----
            def emit_upsample_slab(b, si):
                x = xs[b]
                sl = si * F
                w_val = max(0, min(F, T - sl))
                relb = pp.tile([128, F], dt.float32, tag="relb", name="relb", bufs=2)
                if w_val < F:
                    nc.vector.memset(relb, 0.0)
                if w_val > 0:
                    nc.gpsimd.dma_start(out=relb[:, 0:w_val], in_=bass.AP(
                        tensor=rel_pos.tensor,
                        offset=rel_pos.offset + b * T + sl,
                        ap=[[0, 128], [1, w_val]]))
                u = pp.tile([128, F], dt.float32, tag="u", name="u", bufs=2)
                nc.vector.tensor_scalar(u, relb, fc2pi, coff, Alu.mult, Alu.add)
                kf = pp.tile([128, F], dt.float32, tag="kf", name="kf", bufs=2)
                nc.vector.tensor_scalar(kf, u, 8388608.0, 8388608.0,
                                        Alu.add, Alu.subtract)
                nc.vector.tensor_tensor(u, u, kf, Alu.subtract)
                emb = pp.tile([128, F], dt.bfloat16, tag="emb", name="emb", bufs=2)
                nc.scalar.activation(emb, u, ActF.Sin, bias=zcol, scale=TWO_PI)
                for (pi, a0, a1) in slab_bands[b][si]:
                    ensure_mask(b, pi)
                pst = psts[b]
                xps = psum.tile([C, F], dt.float32, tag="xps", name="xps", bufs=2)
                nb = len(slab_bands[b][si])
                nc.tensor.matmul(xps, posw_bf, emb, start=True, stop=(nb == 0))
                for j, (pi, a0, a1) in enumerate(slab_bands[b][si]):
                    g, lo, _ = chunk_bands[b][pi]
                    nc.tensor.matmul(xps[:, a0 - sl:a1 - sl], pst[:, g, :],
                                     Dg[(b, pi)][:, a0 - lo:a1 - lo],
                                     start=False, stop=(j == nb - 1))
                nc.scalar.activation(x[:, sl:sl + F], xps, ActF.Identity,
                                     bias=ipb_col)

            # ---------- layernorm, slab-pipelined ----------
            epscol = ptile([128, 1], dt.float32, "epscol")
            nc.vector.memset(epscol, EPS)

            stats_bufs = {}

            def emit_stats_slab(b, S1, S2, si):
                # write fp8 copies of x and x^2 into the pair buffer half;
                # on the odd slab, one DoubleRow matmul per stat reduces
                # both slabs at once (contraction 256). Slab halves 0-3 and
                # 4-7 form separate PSUM groups on rows 0-3 of the same
                # tiles so row math never needs a nonzero base partition.
                x = xs[b]
                sl = si * F
                p = si // 2
                q = p % 2
                if si % 2 == 0:
                    stats_bufs[b] = (
                        pp.tile([128, 2, F], dt.float8e4, tag="xbf",
                                name="xbf", bufs=3),
                        pp.tile([128, 2, F], dt.float8e4, tag="xsq",
                                name="xsq", bufs=2))
                xp8, xq8 = stats_bufs[b]
                nc.scalar.activation(xp8[:, si % 2, :], x[:, sl:sl + F],
                                     ActF.Copy)
                nc.vector.tensor_tensor(xq8[:, si % 2, :], x[:, sl:sl + F],
                                        x[:, sl:sl + F], Alu.mult)
                if si % 2 == 1:
                    nc.tensor.matmul(S1, ohdr8[:, q, :, :], xp8,
                                     start=(q == 0), stop=(q == 1),
                                     perf_mode=mybir.MatmulPerfMode.DoubleRow)
                    nc.tensor.matmul(S2, ohdr8[:, q, :, :], xq8,
                                     start=(q == 0), stop=(q == 1),
                                     perf_mode=mybir.MatmulPerfMode.DoubleRow)

            def emit_stats_and_rowmath(key, b, si, strc):
                emit_stats_slab(b, *stats_tiles[key], si)
                if si % 4 == 3:
                    emit_rowmath_rows(*stats_tiles[key], 4, strc,
                                      4 * (si // 4))

            def emit_rowmath_rows(S1, S2, nr, strc, strc_r0):
                # LN row math on stats rows [0, nr); writes packed
                # (rstd, mu*rstd) rows [strc_r0, strc_r0+nr) of the DRAM
                # bounce strc[si, 0:2, F].
                s = slice(0, nr)
                mu = pp.tile([8, F], dt.float32, tag="mu", name="mu", bufs=1)
                nc.vector.tensor_scalar(mu[s], S1[s], 1.0 / C, None, Alu.mult)
                m2 = pp.tile([8, F], dt.float32, tag="mm2", name="mm2", bufs=1)
                nc.vector.scalar_tensor_tensor(m2[s], S1[s], 1.0 / C, mu[s],
                                               Alu.mult, Alu.mult)
                vr = pp.tile([8, F], dt.float32, tag="vr", name="vr", bufs=1)
                nc.vector.scalar_tensor_tensor(vr[s], S2[s], 1.0 / C, m2[s],
                                               Alu.mult, Alu.subtract)
                sq = pp.tile([8, F], dt.float32, tag="sq", name="sq", bufs=1)
                nc.scalar.activation(sq[s], vr[s], ActF.Sqrt, bias=epscol[s])
                rF = pp.tile([8, F], dt.float32, tag="rF", name="rF", bufs=1)
                nc.vector.reciprocal_approx_fast(rF[s], sq[s])
                rFb = pp.tile([8, F], dt.bfloat16, tag="rFb", name="rFb", bufs=1)
                nc.vector.tensor_copy(rFb[s], rF[s])
                cF = pp.tile([8, F], dt.bfloat16, tag="cF", name="cF", bufs=1)
                nc.vector.tensor_tensor(cF[s], mu[s], rF[s], Alu.mult)
                so = slice(strc_r0, strc_r0 + nr)
                nc.scalar.dma_start(out=strc[so, 0, :], in_=rFb[s])
                nc.scalar.dma_start(out=strc[so, 1, :], in_=cF[s])

            def new_strc():
                return dram.tile([8, 2, F], dt.bfloat16, tag="strc", name="strc",
                                 bufs=2)

            def emit_norm_slab(b, si, strc, gcol, bcol, out_act, out_tile, out_off):
                x = xs[b]
                sl = si * F
                bc = pp.tile([128, 2 * F], dt.bfloat16, tag="ABs", name="ABs", bufs=4)
                nc.gpsimd.dma_start(out=bc, in_=bass.AP(
                    tensor=strc.tensor, offset=strc.offset + si * 2 * F,
                    ap=[[0, 128], [1, 2 * F]]))
                w = min(F, T - sl)
                t1 = pp.tile([128, F], dt.float32, tag="t1", name="t1", bufs=4)
                nc.vector.tensor_tensor(t1[:, 0:w], x[:, sl:sl + w],
                                        bc[:, 0:w], Alu.mult)
                nc.vector.tensor_tensor(t1[:, 0:w], t1[:, 0:w],
                                        bc[:, F:F + w], Alu.subtract)
                nc.scalar.activation(out_tile[:, out_off + sl:out_off + sl + w],
                                     t1[:, 0:w], out_act, bias=bcol, scale=gcol)

            def emit_conv_slab(b, l, si):
                x, h = xs[b], hs[b]
                dil = dils[l]
                sl = si * F
                w = min(F, T - sl)
                cv = psum.tile([128, F], dt.float32, tag="cv", name="cv", bufs=2)
                for k in range(K):
                    off = HPAD + sl - (K - 1 - k) * dil
                    nc.tensor.matmul(cv[:, 0:w], cw_bf[l][:, k, :],
                                     h[:, off:off + w],
                                     start=(k == 0), stop=(k == K - 1))
                nc.vector.scalar_tensor_tensor(
                    x[:, sl:sl + w], cv[:, 0:w], conv_b_col[l], x[:, sl:sl + w],
                    Alu.add, Alu.add)

            def emit_out_chunk(b, tchunk):
                ybf = ybfs[b]
                t0 = 128 * tchunk
                nrows = min(128, T - t0)
                if nrows <= 0:
                    return
                pof = psum.tile([128, F], dt.float32, tag="cv", name="cv", bufs=2)
                po = pof[:, 0:DOUT]
                nc.tensor.matmul(po, ybf[:, t0:t0 + 128], outw_bf,
                                 start=True, stop=True)
                ost = pp.tile([128, DOUT], dt.bfloat16, tag="ost", name="ost", bufs=8)
                nc.vector.tensor_tensor(ost, po, outbB, Alu.add)
                qe = (nc.sync, nc.gpsimd, nc.scalar)[tchunk % 3]
                qe.dma_start(out=out[b, t0:t0 + nrows, :], in_=ost[:nrows, :])

            for b in range(BPC):
                emit_cum(b)
            emit_proj(0)

            owner_slab = [min(7, ((tc_ + 1) * 128 - 1) // F) for tc_ in range(NT)]
            stats_tiles = {}
            rcs = {}

            def new_stats(key):
                stats_tiles[key] = (
                    psum.tile([16, F], dt.float32, tag="st", name="st", bufs=4),
                    psum.tile([16, F], dt.float32, tag="st", name="st", bufs=4))

            def norm_args(stage, b):
                if stage < L:
                    return (g_col[stage], b_col[stage], _GELU, hs[b], HPAD)
                return (og_col, ob_col, ActF.Identity, ybfs[b], 0)

            # ---- upsample item 0 + its stats; conv starts right after ----
            new_stats(0)
            rcs[0] = new_strc()
            rcs[1] = new_strc()
            for si in range(8):
                emit_upsample_slab(0, si)
                emit_stats_and_rowmath(0, 0, si, rcs[0])
                if si == 0:
                    emit_deferred_weight_casts()
                    emit_proj(1)

            STEPS = [(0, 0), (1, 0), (0, 1), (1, 1), (0, 2), (1, 2)]
            for step, (b, stage) in enumerate(STEPS):
                na = norm_args(stage, b)
                nxt = STEPS[step + 1] if step + 1 < len(STEPS) else None
                first = step == 0
                last = step == len(STEPS) - 1
                if last:
                    rco = new_strc()      # strc for (0, out)
                    rco1 = new_strc()     # strc for (1, out)
                    new_stats("o1")
                if first:
                    new_stats(1)
                for si in range(8):
                    emit_norm_slab(b, si, rcs[b], *na)
                    emit_conv_slab(b, stage, si)
                    if first:
                        # inject item 1's upsample+stats under item 0's convs
                        emit_upsample_slab(1, si)
                        emit_stats_and_rowmath(1, 1, si, rcs[1])
                    elif not last:
                        if si == 2:
                            rcs[nxt[0]] = new_strc()
                            new_stats(nxt[0])
                        if 2 <= si <= 5:
                            for sj in ((si - 2) * 2, (si - 2) * 2 + 1):
                                emit_stats_and_rowmath(nxt[0], nxt[0], sj,
                                                       rcs[nxt[0]])
                    else:
                        # tail: (0, out) stats under (1, 2)'s convs; item 1's
                        # out-LN stats as each of its conv slabs finishes
                        if si == 2:
                            new_stats("o0")
                        if 2 <= si <= 5:
                            for sj in ((si - 2) * 2, (si - 2) * 2 + 1):
                                emit_stats_and_rowmath("o0", 0, sj, rco)
                        emit_stats_and_rowmath("o1", 1, si, rco1)
                        if si == 7:
                            nao = norm_args(L, 0)
                            oc = 0
                            for sj in range(8):
                                emit_norm_slab(0, sj, rco, *nao)
                                while oc < NT and owner_slab[oc] <= sj:
                                    emit_out_chunk(0, oc)
                                    oc += 1

            # ---- (1, out) ----
            nao = norm_args(L, 1)
            oc = 0
            for si in range(8):
                emit_norm_slab(1, si, rco1, *nao)
                while oc < NT and owner_slab[oc] <= si:
                    emit_out_chunk(1, oc)
                    oc += 1

    nc.compile()
    return nc


_NC_CACHE = {}


def _compute_bands(durations, T):
    """Union-over-cores chunk band pieces (width <= 512) and per-slab
    intersections referencing piece indices."""
    TP = _ceil_to(T, 128)
    F = TP // 8
    NCH = N // 128
    cum = np.cumsum(np.asarray(durations, np.int64), axis=1)
    cumd = np.concatenate([np.zeros((B, 1), np.int64), cum], axis=1)
    chunk_bands, slab_bands = [], []
    for b in range(BPC):
        items = list(range(b, B, BPC))
        pieces = []
        for g in range(NCH):
            lo = int(cumd[items, g * 128].min())
            hi = int(cumd[items, (g + 1) * 128].max())
            while hi > lo:
                mid = min(lo + 512, hi)
                pieces.append((g, lo, mid))
                lo = mid
        slabs = []
        for si in range(8):
            sl = si * F
            ent = []
            for pi, (g, lo, hi) in enumerate(pieces):
                a0 = max(lo, sl)
                a1 = min(hi, sl + F)
                if a1 > a0:
                    ent.append((pi, a0, a1))
            slabs.append(tuple(ent))
        chunk_bands.append(tuple(pieces))
        slab_bands.append(tuple(slabs))
    return tuple(chunk_bands), tuple(slab_bands)


def _get_nc(T, chunk_bands, slab_bands):
    key = (T, chunk_bands, slab_bands)
    if key not in _NC_CACHE:
        _NC_CACHE[key] = build_nc(T, chunk_bands, slab_bands)
    return _NC_CACHE[key]


def make_in_maps(pooled, rel_pos, in_w, in_b, pos_w, pos_b, conv_w, conv_b,
                 ln_g, ln_b, out_ln_g, out_ln_b, out_w, out_b, durations):
    shared = {
        "in_wT": np.ascontiguousarray(np.asarray(in_w, np.float32).T),
        "pos_wT": np.ascontiguousarray(np.asarray(pos_w, np.float32).T),
        "conv_wr": np.ascontiguousarray(np.asarray(conv_w, np.float32).transpose(0, 3, 2, 1)),
        "conv_b": np.asarray(conv_b, np.float32),
        "ln_g": np.asarray(ln_g, np.float32),
        "ln_b": np.asarray(ln_b, np.float32),
        "out_ln_g": np.asarray(out_ln_g, np.float32),
        "out_ln_b": np.asarray(out_ln_b, np.float32),
        "out_wT": np.ascontiguousarray(np.asarray(out_w, np.float32).T),
        "out_b": np.asarray(out_b, np.float32),
        "ipb_row": (np.asarray(in_b, np.float32)
                    + np.asarray(pos_b, np.float32))[None, :],
    }
    dur = np.asarray(durations, np.int64)
    cum = np.cumsum(dur, axis=1)
    cumd_all = np.concatenate([np.zeros((B, 1), np.int64), cum],
                              axis=1).astype(np.float32)
    pooledT_all = np.ascontiguousarray(
        np.asarray(pooled, np.float32).transpose(0, 2, 1)
        .reshape(B, 2, 128, N).transpose(0, 2, 1, 3))
    in_maps = []
    for c in range(NCORES):
        s = slice(c * BPC, (c + 1) * BPC)
        m = dict(shared)
        m["pooledT"] = np.ascontiguousarray(pooledT_all[s])
        m["cumd"] = np.ascontiguousarray(cumd_all[s])
        m["rel_pos"] = np.ascontiguousarray(np.asarray(rel_pos, np.float32)[s])
        in_maps.append(m)
    return in_maps


def kernel(**inputs):
    T = inputs["rel_pos"].shape[1]
    chunk_bands, slab_bands = _compute_bands(inputs["durations"], T)
    nc = _get_nc(T, chunk_bands, slab_bands)
    in_maps = make_in_maps(**inputs)
    res = bass_utils.run_bass_kernel_spmd(nc, in_maps, core_ids=list(range(NCORES)))
    return np.concatenate([np.asarray(res.results[c]["out"])
                           for c in range(NCORES)], axis=0).astype(np.float32)


# revision 75
# speedup vs baseline: 1.0001x; 1.0001x over previous
"""DurationConditioningProjector Trainium2 kernel.

Data-parallel over batch B=16 across 8 NeuronCores (2 items per core).
Host does input relayout only (transpose/cumsum/bias folding); all model
compute runs on-device. The emitted program is specialized to the union
(over cores) of phoneme-chunk/frame-slab intersections derived from the
host-computed cumsum; per-core band masks built on-device from that
core's own cum make the union slack contribute exactly zero, so one
SPMD program serves all cores and stays correct for any input (new
inputs recompile via the cache key).

Per-item layout: residual x as (C=128 partitions, T free) fp32 in SBUF.
- Length-regulator upsample AS MATMUL: per phoneme chunk g a band mask
  D[m,t] = [t>=cum[m-1]] - [t>=cum[m]] is built once over the chunk's
  frame band (2 is_ge + 1 sub against a global iota); each slab's PSUM
  gets one sliced matmul per intersecting chunk, plus the sin/cos
  pos-emb matmul and an in_b+pos_b rank-1.
- LayerNorm stats as (8,F) PSUM rows via one-hot-column matmuls; row
  math on (8,F) tiles; per-frame scale/offset broadcast to (128,F) by
  0-stride DMA from a DRAM bounce; normalize = 2 DVE TTs + fused ACT
  gelu(g*z+b).
- 3 dilated causal conv layers: 31 shifted bf16 matmuls per slab into
  PSUM; residual add fused in one scalar_tensor_tensor. Slab-level
  software pipelining: item 1's upsample and each step's stats are
  injected into the running conv stream so the PE never drains.
"""
import sys
sys.path.insert(0, '/opt/trn_rl_repo')

import math
import os
import numpy as np

import concourse.bass as bass
import concourse.mybir as mybir
import concourse.tile as tile
from concourse import bacc
from concourse import bass_utils

dt = mybir.dt
Alu = mybir.AluOpType
ActF = mybir.ActivationFunctionType
_GELU = ActF.Tanh if os.environ.get('KSIM_TANH') else ActF.Gelu

B, N, DIN, C, DOUT, K, L = 16, 1024, 256, 128, 256, 31, 3
NCORES = 8
BPC = B // NCORES
TWO_PI = 2.0 * math.pi
EPS = 1e-5


def _ceil_to(x, m):
    return (x + m - 1) // m * m


def build_nc(T, chunk_bands, slab_bands):
    # chunk_bands[b] = tuple of (g, lo, hi) mask pieces (width <= 512)
    # slab_bands[b][si] = tuple of (piece_idx, a0, a1): piece contributes
    # to absolute frames [a0, a1) within slab si.
    TP = _ceil_to(T, 128)
    NT = TP // 128
    F = TP // 8
    assert F % 16 == 0 and F <= 512
    NCH = N // 128
    WG = 32 * ((max((hi - lo) for cb in chunk_bands
                    for (_, lo, hi) in cb) + 31) // 32)
    NPIECE = sum(len(cb) for cb in chunk_bands)

    nc = bacc.Bacc("TRN2", target_bir_lowering=False, debug=False)

    pooledT = nc.dram_tensor("pooledT", [BPC, 128, 2, N], dt.float32, kind="ExternalInput").ap()
    cumd = nc.dram_tensor("cumd", [BPC, N + 1], dt.float32, kind="ExternalInput").ap()
    rel_pos = nc.dram_tensor("rel_pos", [BPC, T], dt.float32, kind="ExternalInput").ap()
    in_wT = nc.dram_tensor("in_wT", [DIN, C], dt.float32, kind="ExternalInput").ap()
    pos_wT = nc.dram_tensor("pos_wT", [C, C], dt.float32, kind="ExternalInput").ap()
    conv_wr = nc.dram_tensor("conv_wr", [L, K, C, C], dt.float32, kind="ExternalInput").ap()
    conv_b = nc.dram_tensor("conv_b", [L, C], dt.float32, kind="ExternalInput").ap()
    ln_g = nc.dram_tensor("ln_g", [L, C], dt.float32, kind="ExternalInput").ap()
    ln_b = nc.dram_tensor("ln_b", [L, C], dt.float32, kind="ExternalInput").ap()
    out_ln_g = nc.dram_tensor("out_ln_g", [C], dt.float32, kind="ExternalInput").ap()
    out_ln_b = nc.dram_tensor("out_ln_b", [C], dt.float32, kind="ExternalInput").ap()
    out_wT = nc.dram_tensor("out_wT", [C, DOUT], dt.float32, kind="ExternalInput").ap()
    out_b = nc.dram_tensor("out_b", [DOUT], dt.float32, kind="ExternalInput").ap()
    ipb_row = nc.dram_tensor("ipb_row", [1, C], dt.float32, kind="ExternalInput").ap()
    out = nc.dram_tensor("out", [BPC, T, DOUT], dt.bfloat16, kind="ExternalOutput").ap()

    iota_c = nc.inline_tensor(
        np.broadcast_to(np.arange(512, dtype=np.float32), (128, 512)).copy(), "iotac")
    oh = np.zeros((8, 8), np.float32)
    np.fill_diagonal(oh, 1.0)
    onehot_c = nc.inline_tensor(
        np.broadcast_to(oh[None, :, :], (128, 8, 8)).copy(), "onehotc")
    ohdr = np.zeros((2, 2, 16), np.float32)
    for q_ in range(2):
        ohdr[q_, 0, 2 * q_] = 1.0
        ohdr[q_, 1, 2 * q_ + 1] = 1.0
    ohdr_c = nc.inline_tensor(
        np.broadcast_to(ohdr[None], (128, 2, 2, 16)).copy(), "ohdrc")
    half = C // 2
    freqs = np.exp(-math.log(10000.0) * np.arange(half, dtype=np.float64) / (half - 1))
    fc2pi_c = nc.inline_tensor(
        (np.concatenate([freqs, freqs]) / TWO_PI).astype(np.float32)[:, None], "fc2pic")
    coff_np = np.zeros((C, 1), np.float32)
    coff_np[half:] = 0.25
    coff_c = nc.inline_tensor(coff_np, "coffc")

    dils = [2 ** i for i in range(L)]
    HPAD = (K - 1) * dils[-1]

    with tile.TileContext(nc) as tc:
        import contextlib
        ctx = contextlib.ExitStack()
        with ctx:
            pers = ctx.enter_context(tc.tile_pool(name="pers", bufs=1))
            pp = ctx.enter_context(tc.tile_pool(name="pp", bufs=2))
            psum = ctx.enter_context(tc.tile_pool(name="ps", bufs=1, space="PSUM"))
            dram = ctx.enter_context(tc.tile_pool(name="dr", bufs=1, space="DRAM"))

            def ptile(shape, d, nm, bufs=1, pool=None):
                pool = pool or pers
                return pool.tile(shape, d, tag=nm, name=nm, bufs=bufs)

            # ---------- shared setup ----------
            iot512 = ptile([128, 512], dt.float32, "iot512")
            nc.sync.dma_start(out=iot512, in_=iota_c.ap())
            fc2pi = ptile([C, 1], dt.float32, "fc2pi")
            nc.scalar.dma_start(out=fc2pi, in_=fc2pi_c.ap())
            coff = ptile([C, 1], dt.float32, "coff")
            nc.scalar.dma_start(out=coff, in_=coff_c.ap())
            onesF = ptile([1, F], dt.bfloat16, "onesF")
            nc.vector.memset(onesF, 1.0)
            ohdr_f = ptile([128, 2, 2, 16], dt.float32, "ohdrf", pool=pp)
            nc.scalar.dma_start(out=ohdr_f, in_=ohdr_c.ap())
            ohdr8 = ptile([128, 2, 2, 16], dt.float8e4, "ohdr8")
            nc.vector.tensor_copy(ohdr8, ohdr_f)
            zcol = ptile([128, 1], dt.float32, "zcol")
            nc.vector.memset(zcol, 0.0)

            def col(src_1d, nm):
                t = ptile([C, 1], dt.float32, nm)
                nc.scalar.dma_start(out=t, in_=src_1d[:, None])
                return t

            ipb_col = ptile([C, 1], dt.float32, "ipbcol")
            nc.scalar.dma_start(out=ipb_col, in_=bass.AP(
                tensor=ipb_row.tensor, offset=ipb_row.offset, ap=[[1, C], [C, 1]]))
            conv_b_col = [col(conv_b[l], f"cb{l}") for l in range(L)]
            g_col = [col(ln_g[l], f"g{l}") for l in range(L)]
            b_col = [col(ln_b[l], f"b{l}") for l in range(L)]
            og_col = col(out_ln_g, "og")
            ob_col = col(out_ln_b, "ob")

            outbB = ptile([128, DOUT], dt.float32, "outbB")
            nc.scalar.dma_start(out=outbB, in_=bass.AP(
                tensor=out_b.tensor, offset=out_b.offset, ap=[[0, 128], [1, DOUT]]))

            inw_f = ptile([128, 2, C], dt.float32, "inwf")
            nc.scalar.dma_start(out=inw_f, in_=in_wT.rearrange("(a p) c -> p a c", p=128))
            inw_bf = ptile([128, 2, C], dt.bfloat16, "inwbf")
            nc.vector.tensor_copy(inw_bf, inw_f)
            posw_f = ptile([C, C], dt.float32, "poswf")
            nc.scalar.dma_start(out=posw_f, in_=pos_wT)
            posw_bf = ptile([C, C], dt.bfloat16, "poswbf")
            nc.vector.tensor_copy(posw_bf, posw_f)
            outw_f = ptile([C, DOUT], dt.float32, "outwf")
            nc.scalar.dma_start(out=outw_f, in_=out_wT)
            outw_bf = ptile([C, DOUT], dt.bfloat16, "outwbf")
            cw_bf = [ptile([128, K, C], dt.bfloat16, f"cw{l}")
                     for l in range(L)]

            def emit_deferred_weight_casts():
                # conv-weight staging + bf16 conversions queued only after
                # the upsample chain's ops, so the startup critical path
                # (DVE in-order queue) is not delayed
                nc.vector.tensor_copy(outw_bf, outw_f)
                for l in range(L):
                    cwf = ptile([128, K, C], dt.float32, "cwstage", pool=pp,
                                bufs=1)
                    (nc.scalar, nc.sync, nc.gpsimd)[l].dma_start(
                        out=cwf, in_=conv_wr[l].rearrange("k p c -> p k c"))
                    nc.vector.tensor_copy(cw_bf[l], cwf)

            # ---------- per-item persistent ----------
            xs, hs = [], []
            for b in range(BPC):
                xs.append(ptile([C, TP], dt.float32, f"x{b}"))
                h = ptile([C, HPAD + TP], dt.bfloat16, f"h{b}")
                nc.vector.memset(h[:, 0:HPAD], 0.0)
                hs.append(h)
            ybfs = [ptile([C, TP], dt.bfloat16, f"ybf{b}") for b in range(BPC)]

            # ---------- cum band edges (host-computed cumd) ----------
            cumws, cumw0s = [], []
            def emit_cum(b):
                cumw = ptile([128, NCH], dt.float32, f"cumw{b}", pool=pp)
                nc.sync.dma_start(out=cumw, in_=bass.AP(
                    tensor=cumd.tensor, offset=cumd.offset + b * (N + 1) + 1,
                    ap=[[1, 128], [128, NCH]]))
                cumw0 = ptile([128, NCH], dt.float32, f"cumw0{b}", pool=pp)
                nc.sync.dma_start(out=cumw0, in_=bass.AP(
                    tensor=cumd.tensor, offset=cumd.offset + b * (N + 1),
                    ap=[[1, 128], [128, NCH]]))
                cumws.append(cumw)
                cumw0s.append(cumw0)

            # ---------- per-chunk band masks ----------
            # D[m, t] = [t >= cum[m-1]] - [t >= cum[m]] over the chunk's
            # band; all masks depend only on iotG + cum so they pipeline
            # far ahead of the PE.
            Dg = {}
            def ensure_mask(b, pi):
                # lazily build D = [t>=cum0]*[t<cum1] (2 DVE ops) so mask
                # work interleaves with the consuming slab instead of
                # front-loading the vector queue
                if (b, pi) in Dg:
                    return Dg[(b, pi)]
                g, lo, hi = chunk_bands[b][pi]
                w = hi - lo
                bv0p = pp.tile([128, 1], dt.float32, tag="bvp", name="bvp",
                               bufs=4)
                nc.vector.tensor_scalar(bv0p, cumw0s[b][:, g:g + 1], float(lo),
                                        None, Alu.subtract)
                bv1p = pp.tile([128, 1], dt.float32, tag="bvp", name="bvp",
                               bufs=4)
                nc.vector.tensor_scalar(bv1p, cumws[b][:, g:g + 1], float(lo),
                                        None, Alu.subtract)
                S0 = pp.tile([128, WG], dt.bfloat16, tag="S0", name="S0", bufs=2)
                nc.vector.tensor_scalar(S0[:, 0:w], iot512[:, 0:w],
                                        bv0p, None, Alu.is_ge)
                D = pp.tile([128, WG], dt.bfloat16, tag="Dg", name="Dg",
                            bufs=NPIECE)
                nc.vector.scalar_tensor_tensor(D[:, 0:w], iot512[:, 0:w],
                                               bv1p, S0[:, 0:w],
                                               Alu.is_lt, Alu.mult)
                Dg[(b, pi)] = D
                return D

            # ---------- P'' projection (SBUF-resident, bf16) ----------
            psts = []
            def emit_proj(b):
                ptf = pp.tile([128, 2, N], dt.float32, tag="ptf", name="ptf", bufs=1)
                src = pooledT[b]
                ptb = ptile([128, 2, N], dt.bfloat16, f"ptb{b}", pool=pp, bufs=1)
                for qi, qe in enumerate((nc.sync, nc.scalar, nc.gpsimd, nc.sync)):
                    qe.dma_start(out=ptf[:, :, qi * 256:(qi + 1) * 256],
                                 in_=src[:, :, qi * 256:(qi + 1) * 256])
                    nc.vector.tensor_copy(ptb[:, :, qi * 256:(qi + 1) * 256],
                                          ptf[:, :, qi * 256:(qi + 1) * 256])
                pst = ptile([128, NCH, C], dt.bfloat16, f"pst{b}")
                for nchunk in range(NCH):
                    ppsf = psum.tile([128, F], dt.float32, tag="xps", name="xps", bufs=2)
                    pps = ppsf[:, 0:C]
                    for dh in range(2):
                        nc.tensor.matmul(pps, ptb[:, dh, 128 * nchunk:128 * (nchunk + 1)],
                                         inw_bf[:, dh, :], start=(dh == 0), stop=(dh == 1))
                    nc.vector.tensor_copy(pst[:, nchunk, :], pps)
                psts.append(pst)

            # ---------- upsample + posemb, one PSUM accumulation per slab ----
            def emit_upsample_slab(b, si):
                x = xs[b]
                sl = si * F
                w_val = max(0, min(F, T - sl))
                relb = pp.tile([128, F], dt.float32, tag="relb", name="relb", bufs=2)
                if w_val < F:
                    nc.vector.memset(relb, 0.0)
                if w_val > 0:
                    nc.gpsimd.dma_start(out=relb[:, 0:w_val], in_=bass.AP(
                        tensor=rel_pos.tensor,
                        offset=rel_pos.offset + b * T + sl,
                        ap=[[0, 128], [1, w_val]]))
                u = pp.tile([128, F], dt.float32, tag="u", name="u", bufs=2)
                nc.vector.tensor_scalar(u, relb, fc2pi, coff, Alu.mult, Alu.add)
                kf = pp.tile([128, F], dt.float32, tag="kf", name="kf", bufs=2)
                nc.vector.tensor_scalar(kf, u, 8388608.0, 8388608.0,
                                        Alu.add, Alu.subtract)
                nc.vector.tensor_tensor(u, u, kf, Alu.subtract)
                emb = pp.tile([128, F], dt.bfloat16, tag="emb", name="emb", bufs=2)
                nc.scalar.activation(emb, u, ActF.Sin, bias=zcol, scale=TWO_PI)
                for (pi, a0, a1) in slab_bands[b][si]:
                    ensure_mask(b, pi)
                pst = psts[b]
                xps = psum.tile([C, F], dt.float32, tag="xps", name="xps", bufs=2)
                nb = len(slab_bands[b][si])
                nc.tensor.matmul(xps, posw_bf, emb, start=True, stop=(nb == 0))
                for j, (pi, a0, a1) in enumerate(slab_bands[b][si]):
                    g, lo, _ = chunk_bands[b][pi]
                    nc.tensor.matmul(xps[:, a0 - sl:a1 - sl], pst[:, g, :],
                                     Dg[(b, pi)][:, a0 - lo:a1 - lo],
                                     start=False, stop=(j == nb - 1))
                nc.scalar.activation(x[:, sl:sl + F], xps, ActF.Identity,
                                     bias=ipb_col)

            # ---------- layernorm, slab-pipelined ----------
            epscol = ptile([128, 1], dt.float32, "epscol")
            nc.vector.memset(epscol, EPS)

            stats_bufs = {}

            def emit_stats_slab(b, S1, S2, si):
                # write fp8 copies of x and x^2 into the pair buffer half;
                # on the odd slab, one DoubleRow matmul per stat reduces
                # both slabs at once (contraction 256). Slab halves 0-3 and
                # 4-7 form separate PSUM groups on rows 0-3 of the same
                # tiles so row math never needs a nonzero base partition.
                x = xs[b]
                sl = si * F
                p = si // 2
                q = p % 2
                if si % 2 == 0:
                    stats_bufs[b] = (
                        pp.tile([128, 2, F], dt.float8e4, tag="xbf",
                                name="xbf", bufs=2),
                        pp.tile([128, 2, F], dt.float8e4, tag="xsq",
                                name="xsq", bufs=2))
                xp8, xq8 = stats_bufs[b]
                nc.scalar.activation(xp8[:, si % 2, :], x[:, sl:sl + F],
                                     ActF.Copy)
                nc.vector.tensor_tensor(xq8[:, si % 2, :], x[:, sl:sl + F],
                                        x[:, sl:sl + F], Alu.mult)
                if si % 2 == 1:
                    nc.tensor.matmul(S1, ohdr8[:, q, :, :], xp8,
                                     start=(q == 0), stop=(q == 1),
                                     perf_mode=mybir.MatmulPerfMode.DoubleRow)
                    nc.tensor.matmul(S2, ohdr8[:, q, :, :], xq8,
                                     start=(q == 0), stop=(q == 1),
                                     perf_mode=mybir.MatmulPerfMode.DoubleRow)

            def emit_stats_and_rowmath(key, b, si, strc):
                emit_stats_slab(b, *stats_tiles[key], si)
                if si % 4 == 3:
                    emit_rowmath_rows(*stats_tiles[key], 4, strc,
                                      4 * (si // 4))

            def emit_rowmath_rows(S1, S2, nr, strc, strc_r0):
                # LN row math on stats rows [0, nr); writes packed
                # (rstd, mu*rstd) rows [strc_r0, strc_r0+nr) of the DRAM
                # bounce strc[si, 0:2, F].
                s = slice(0, nr)
                mu = pp.tile([8, F], dt.float32, tag="mu", name="mu", bufs=1)
                nc.vector.tensor_scalar(mu[s], S1[s], 1.0 / C, None, Alu.mult)
                m2 = pp.tile([8, F], dt.float32, tag="mm2", name="mm2", bufs=1)
                nc.vector.scalar_tensor_tensor(m2[s], S1[s], 1.0 / C, mu[s],
                                               Alu.mult, Alu.mult)
                vr = pp.tile([8, F], dt.float32, tag="vr", name="vr", bufs=1)
                nc.vector.scalar_tensor_tensor(vr[s], S2[s], 1.0 / C, m2[s],
                                               Alu.mult, Alu.subtract)
                sq = pp.tile([8, F], dt.float32, tag="sq", name="sq", bufs=1)
                nc.scalar.activation(sq[s], vr[s], ActF.Sqrt, bias=epscol[s])
                rF = pp.tile([8, F], dt.float32, tag="rF", name="rF", bufs=1)
                nc.vector.reciprocal_approx_fast(rF[s], sq[s])
                rFb = pp.tile([8, F], dt.bfloat16, tag="rFb", name="rFb", bufs=1)
                nc.vector.tensor_copy(rFb[s], rF[s])
                cF = pp.tile([8, F], dt.bfloat16, tag="cF", name="cF", bufs=1)
                nc.vector.tensor_tensor(cF[s], mu[s], rF[s], Alu.mult)
                so = slice(strc_r0, strc_r0 + nr)
                nc.scalar.dma_start(out=strc[so, 0, :], in_=rFb[s])
                nc.scalar.dma_start(out=strc[so, 1, :], in_=cF[s])

            def new_strc():
                return dram.tile([8, 2, F], dt.bfloat16, tag="strc", name="strc",
                                 bufs=2)

            def emit_norm_slab(b, si, strc, gcol, bcol, out_act, out_tile, out_off):
                x = xs[b]
                sl = si * F
                bc = pp.tile([128, 2 * F], dt.bfloat16, tag="ABs", name="ABs", bufs=4)
                nc.gpsimd.dma_start(out=bc, in_=bass.AP(
                    tensor=strc.tensor, offset=strc.offset + si * 2 * F,
                    ap=[[0, 128], [1, 2 * F]]))
                w = min(F, T - sl)
                t1 = pp.tile([128, F], dt.float32, tag="t1", name="t1", bufs=3)
                nc.vector.tensor_tensor(t1[:, 0:w], x[:, sl:sl + w],
                                        bc[:, 0:w], Alu.mult)
                nc.vector.tensor_tensor(t1[:, 0:w], t1[:, 0:w],
                                        bc[:, F:F + w], Alu.subtract)
                nc.scalar.activation(out_tile[:, out_off + sl:out_off + sl + w],
                                     t1[:, 0:w], out_act, bias=bcol, scale=gcol)

            def emit_conv_slab(b, l, si):
                x, h = xs[b], hs[b]
                dil = dils[l]
                sl = si * F
                w = min(F, T - sl)
                cv = psum.tile([128, F], dt.float32, tag="cv", name="cv", bufs=2)
                for k in range(K):
                    off = HPAD + sl - (K - 1 - k) * dil
                    nc.tensor.matmul(cv[:, 0:w], cw_bf[l][:, k, :],
                                     h[:, off:off + w],
                                     start=(k == 0), stop=(k == K - 1))
                nc.vector.scalar_tensor_tensor(
                    x[:, sl:sl + w], cv[:, 0:w], conv_b_col[l], x[:, sl:sl + w],
                    Alu.add, Alu.add)

            def emit_out_chunk(b, tchunk):
                ybf = ybfs[b]
                t0 = 128 * tchunk
                nrows = min(128, T - t0)
                if nrows <= 0:
                    return
                pof = psum.tile([128, F], dt.float32, tag="cv", name="cv", bufs=2)
                po = pof[:, 0:DOUT]
                nc.tensor.matmul(po, ybf[:, t0:t0 + 128], outw_bf,
                                 start=True, stop=True)
                ost = pp.tile([128, DOUT], dt.bfloat16, tag="ost", name="ost", bufs=6)
                nc.vector.tensor_tensor(ost, po, outbB, Alu.add)
                qe = (nc.sync, nc.gpsimd, nc.scalar)[tchunk % 3]
                qe.dma_start(out=out[b, t0:t0 + nrows, :], in_=ost[:nrows, :])

            for b in range(BPC):
                emit_cum(b)
            emit_proj(0)

            owner_slab = [min(7, ((tc_ + 1) * 128 - 1) // F) for tc_ in range(NT)]
            stats_tiles = {}
            rcs = {}

            def new_stats(key):
                stats_tiles[key] = (
                    psum.tile([16, F], dt.float32, tag="st", name="st", bufs=4),
                    psum.tile([16, F], dt.float32, tag="st", name="st", bufs=4))

            def norm_args(stage, b):
                if stage < L:
                    return (g_col[stage], b_col[stage], _GELU, hs[b], HPAD)
                return (og_col, ob_col, ActF.Identity, ybfs[b], 0)

            # ---- upsample item 0 + its stats; conv starts right after ----
            new_stats(0)
            rcs[0] = new_strc()
            rcs[1] = new_strc()
            for si in range(8):
                emit_upsample_slab(0, si)
                emit_stats_and_rowmath(0, 0, si, rcs[0])
                if si == 0:
                    emit_deferred_weight_casts()
                    emit_proj(1)

            STEPS = [(0, 0), (1, 0), (0, 1), (1, 1), (0, 2), (1, 2)]
            for step, (b, stage) in enumerate(STEPS):
                na = norm_args(stage, b)
                nxt = STEPS[step + 1] if step + 1 < len(STEPS) else None
                first = step == 0
                last = step == len(STEPS) - 1
                if last:
                    rco = new_strc()      # strc for (0, out)
                    rco1 = new_strc()     # strc for (1, out)
                    new_stats("o1")
                if first:
                    new_stats(1)
                for si in range(8):
                    emit_norm_slab(b, si, rcs[b], *na)
                    emit_conv_slab(b, stage, si)
                    if first:
                        # inject item 1's upsample+stats under item 0's convs
                        emit_upsample_slab(1, si)
                        emit_stats_and_rowmath(1, 1, si, rcs[1])
                    elif not last:
                        if si == 2:
                            rcs[nxt[0]] = new_strc()
                            new_stats(nxt[0])
                        if 2 <= si <= 5:
                            for sj in ((si - 2) * 2, (si - 2) * 2 + 1):
                                emit_stats_and_rowmath(nxt[0], nxt[0], sj,
                                                       rcs[nxt[0]])
                    else:
                        # tail: (0, out) stats under (1, 2)'s convs; item 1's
                        # out-LN stats as each of its conv slabs finishes
                        if si == 2:
                            new_stats("o0")
                        if 2 <= si <= 5:
                            for sj in ((si - 2) * 2, (si - 2) * 2 + 1):
                                emit_stats_and_rowmath("o0", 0, sj, rco)
                        emit_stats_and_rowmath("o1", 1, si, rco1)
                        if si == 7:
                            nao = norm_args(L, 0)
                            oc = 0
                            for sj in range(8):
                                emit_norm_slab(0, sj, rco, *nao)
                                while oc < NT and owner_slab[oc] <= sj:
                                    emit_out_chunk(0, oc)
                                    oc += 1

            # ---- (1, out) ----
            nao = norm_args(L, 1)
            oc = 0
            for si in range(8):
                emit_norm_slab(1, si, rco1, *nao)
                while oc < NT and owner_slab[oc] <= si:
                    emit_out_chunk(1, oc)
                    oc += 1

    nc.compile()
    return nc


_NC_CACHE = {}


def _compute_bands(durations, T):
    """Union-over-cores chunk band pieces (width <= 512) and per-slab
    intersections referencing piece indices."""
    TP = _ceil_to(T, 128)
    F = TP // 8
    NCH = N // 128
    cum = np.cumsum(np.asarray(durations, np.int64), axis=1)
    cumd = np.concatenate([np.zeros((B, 1), np.int64), cum], axis=1)
    chunk_bands, slab_bands = [], []
    for b in range(BPC):
        items = list(range(b, B, BPC))
        pieces = []
        for g in range(NCH):
            lo = int(cumd[items, g * 128].min())
            hi = int(cumd[items, (g + 1) * 128].max())
            while hi > lo:
                mid = min(lo + 512, hi)
                pieces.append((g, lo, mid))
                lo = mid
        slabs = []
        for si in range(8):
            sl = si * F
            ent = []
            for pi, (g, lo, hi) in enumerate(pieces):
                a0 = max(lo, sl)
                a1 = min(hi, sl + F)
                if a1 > a0:
                    ent.append((pi, a0, a1))
            slabs.append(tuple(ent))
        chunk_bands.append(tuple(pieces))
        slab_bands.append(tuple(slabs))
    return tuple(chunk_bands), tuple(slab_bands)


def _get_nc(T, chunk_bands, slab_bands):
    key = (T, chunk_bands, slab_bands)
    if key not in _NC_CACHE:
        _NC_CACHE[key] = build_nc(T, chunk_bands, slab_bands)
    return _NC_CACHE[key]


def make_in_maps(pooled, rel_pos, in_w, in_b, pos_w, pos_b, conv_w, conv_b,
                 ln_g, ln_b, out_ln_g, out_ln_b, out_w, out_b, durations):
    shared = {
        "in_wT": np.ascontiguousarray(np.asarray(in_w, np.float32).T),
        "pos_wT": np.ascontiguousarray(np.asarray(pos_w, np.float32).T),
        "conv_wr": np.ascontiguousarray(np.asarray(conv_w, np.float32).transpose(0, 3, 2, 1)),
        "conv_b": np.asarray(conv_b, np.float32),
        "ln_g": np.asarray(ln_g, np.float32),
        "ln_b": np.asarray(ln_b, np.float32),
        "out_ln_g": np.asarray(out_ln_g, np.float32),
        "out_ln_b": np.asarray(out_ln_b, np.float32),
        "out_wT": np.ascontiguousarray(np.asarray(out_w, np.float32).T),
        "out_b": np.asarray(out_b, np.float32),
        "ipb_row": (np.asarray(in_b, np.float32)
                    + np.asarray(pos_b, np.float32))[None, :],
    }
    dur = np.asarray(durations, np.int64)
    cum = np.cumsum(dur, axis=1)
    cumd_all = np.concatenate([np.zeros((B, 1), np.int64), cum],
                              axis=1).astype(np.float32)
    pooledT_all = np.ascontiguousarray(
        np.asarray(pooled, np.float32).transpose(0, 2, 1)
        .reshape(B, 2, 128, N).transpose(0, 2, 1, 3))
    in_maps = []
    for c in range(NCORES):
        s = slice(c * BPC, (c + 1) * BPC)
        m = dict(shared)
        m["pooledT"] = np.ascontiguousarray(pooledT_all[s])
        m["cumd"] = np.ascontiguousarray(cumd_all[s])
        m["rel_pos"] = np.ascontiguousarray(np.asarray(rel_pos, np.float32)[s])
        in_maps.append(m)
    return in_maps


def kernel(**inputs):
    T = inputs["rel_pos"].shape[1]
    chunk_bands, slab_bands = _compute_bands(inputs["durations"], T)
    nc = _get_nc(T, chunk_bands, slab_bands)
    in_maps = make_in_maps(**inputs)
    res = bass_utils.run_bass_kernel_spmd(nc, in_maps, core_ids=list(range(NCORES)))
    return np.concatenate([np.asarray(res.results[c]["out"])
                           for c in range(NCORES)], axis=0).astype(np.float32)


# revision 77
# speedup vs baseline: 1.0006x; 1.0005x over previous
"""DurationConditioningProjector Trainium2 kernel.

Data-parallel over batch B=16 across 8 NeuronCores (2 items per core).
Host does input relayout only (transpose/cumsum/bias folding); all model
compute runs on-device. The emitted program is specialized to the union
(over cores) of phoneme-chunk/frame-slab intersections derived from the
host-computed cumsum; per-core band masks built on-device from that
core's own cum make the union slack contribute exactly zero, so one
SPMD program serves all cores and stays correct for any input (new
inputs recompile via the cache key).

Per-item layout: residual x as (C=128 partitions, T free) fp32 in SBUF.
- Length-regulator upsample AS MATMUL: per phoneme chunk g a band mask
  D[m,t] = [t>=cum[m-1]] - [t>=cum[m]] is built once over the chunk's
  frame band (2 is_ge + 1 sub against a global iota); each slab's PSUM
  gets one sliced matmul per intersecting chunk, plus the sin/cos
  pos-emb matmul and an in_b+pos_b rank-1.
- LayerNorm stats as (8,F) PSUM rows via one-hot-column matmuls; row
  math on (8,F) tiles; per-frame scale/offset broadcast to (128,F) by
  0-stride DMA from a DRAM bounce; normalize = 2 DVE TTs + fused ACT
  gelu(g*z+b).
- 3 dilated causal conv layers: 31 shifted bf16 matmuls per slab into
  PSUM; residual add fused in one scalar_tensor_tensor. Slab-level
  software pipelining: item 1's upsample and each step's stats are
  injected into the running conv stream so the PE never drains.
"""
import sys
sys.path.insert(0, '/opt/trn_rl_repo')

import math
import os
import numpy as np

import concourse.bass as bass
import concourse.mybir as mybir
import concourse.tile as tile
from concourse import bacc
from concourse import bass_utils

dt = mybir.dt
Alu = mybir.AluOpType
ActF = mybir.ActivationFunctionType
_GELU = ActF.Tanh if os.environ.get('KSIM_TANH') else ActF.Gelu

B, N, DIN, C, DOUT, K, L = 16, 1024, 256, 128, 256, 31, 3
NCORES = 8
BPC = B // NCORES
TWO_PI = 2.0 * math.pi
EPS = 1e-5


def _ceil_to(x, m):
    return (x + m - 1) // m * m


def build_nc(T, chunk_bands, slab_bands):
    # chunk_bands[b] = tuple of (g, lo, hi) mask pieces (width <= 512)
    # slab_bands[b][si] = tuple of (piece_idx, a0, a1): piece contributes
    # to absolute frames [a0, a1) within slab si.
    TP = _ceil_to(T, 128)
    NT = TP // 128
    F = TP // 8
    assert F % 16 == 0 and F <= 512
    NCH = N // 128
    WG = 32 * ((max((hi - lo) for cb in chunk_bands
                    for (_, lo, hi) in cb) + 31) // 32)
    NPIECE = sum(len(cb) for cb in chunk_bands)

    nc = bacc.Bacc("TRN2", target_bir_lowering=False, debug=False)

    pooledT = nc.dram_tensor("pooledT", [BPC, 128, 2, N], dt.float32, kind="ExternalInput").ap()
    cumd = nc.dram_tensor("cumd", [BPC, N + 1], dt.float32, kind="ExternalInput").ap()
    rel_pos = nc.dram_tensor("rel_pos", [BPC, T], dt.float32, kind="ExternalInput").ap()
    in_wT = nc.dram_tensor("in_wT", [DIN, C], dt.float32, kind="ExternalInput").ap()
    pos_wT = nc.dram_tensor("pos_wT", [C, C], dt.float32, kind="ExternalInput").ap()
    conv_wr = nc.dram_tensor("conv_wr", [L, K, C, C], dt.float32, kind="ExternalInput").ap()
    conv_b = nc.dram_tensor("conv_b", [L, C], dt.float32, kind="ExternalInput").ap()
    ln_g = nc.dram_tensor("ln_g", [L, C], dt.float32, kind="ExternalInput").ap()
    ln_b = nc.dram_tensor("ln_b", [L, C], dt.float32, kind="ExternalInput").ap()
    out_ln_g = nc.dram_tensor("out_ln_g", [C], dt.float32, kind="ExternalInput").ap()
    out_ln_b = nc.dram_tensor("out_ln_b", [C], dt.float32, kind="ExternalInput").ap()
    out_wT = nc.dram_tensor("out_wT", [C, DOUT], dt.float32, kind="ExternalInput").ap()
    out_b = nc.dram_tensor("out_b", [DOUT], dt.float32, kind="ExternalInput").ap()
    ipb_row = nc.dram_tensor("ipb_row", [1, C], dt.float32, kind="ExternalInput").ap()
    out = nc.dram_tensor("out", [BPC, T, DOUT], dt.bfloat16, kind="ExternalOutput").ap()

    iota_c = nc.inline_tensor(
        np.broadcast_to(np.arange(512, dtype=np.float32), (128, 512)).copy(), "iotac")
    oh = np.zeros((8, 8), np.float32)
    np.fill_diagonal(oh, 1.0)
    onehot_c = nc.inline_tensor(
        np.broadcast_to(oh[None, :, :], (128, 8, 8)).copy(), "onehotc")
    ohdr = np.zeros((2, 2, 16), np.float32)
    for q_ in range(2):
        ohdr[q_, 0, 2 * q_] = 1.0
        ohdr[q_, 1, 2 * q_ + 1] = 1.0
    ohdr_c = nc.inline_tensor(
        np.broadcast_to(ohdr[None], (128, 2, 2, 16)).copy(), "ohdrc")
    half = C // 2
    freqs = np.exp(-math.log(10000.0) * np.arange(half, dtype=np.float64) / (half - 1))
    fc2pi_c = nc.inline_tensor(
        (np.concatenate([freqs, freqs]) / TWO_PI).astype(np.float32)[:, None], "fc2pic")
    coff_np = np.zeros((C, 1), np.float32)
    coff_np[half:] = 0.25
    coff_c = nc.inline_tensor(coff_np, "coffc")

    dils = [2 ** i for i in range(L)]
    HPAD = (K - 1) * dils[-1]

    with tile.TileContext(nc) as tc:
        import contextlib
        ctx = contextlib.ExitStack()
        with ctx:
            pers = ctx.enter_context(tc.tile_pool(name="pers", bufs=1))
            pp = ctx.enter_context(tc.tile_pool(name="pp", bufs=2))
            psum = ctx.enter_context(tc.tile_pool(name="ps", bufs=1, space="PSUM"))
            dram = ctx.enter_context(tc.tile_pool(name="dr", bufs=1, space="DRAM"))

            def ptile(shape, d, nm, bufs=1, pool=None):
                pool = pool or pers
                return pool.tile(shape, d, tag=nm, name=nm, bufs=bufs)

            # ---------- shared setup ----------
            iot512 = ptile([128, 512], dt.float32, "iot512")
            nc.sync.dma_start(out=iot512, in_=iota_c.ap())
            fc2pi = ptile([C, 1], dt.float32, "fc2pi")
            nc.scalar.dma_start(out=fc2pi, in_=fc2pi_c.ap())
            coff = ptile([C, 1], dt.float32, "coff")
            nc.scalar.dma_start(out=coff, in_=coff_c.ap())
            onesF = ptile([1, F], dt.bfloat16, "onesF")
            nc.vector.memset(onesF, 1.0)
            ohdr_f = ptile([128, 2, 2, 16], dt.float32, "ohdrf", pool=pp)
            nc.scalar.dma_start(out=ohdr_f, in_=ohdr_c.ap())
            ohdr8 = ptile([128, 2, 2, 16], dt.float8e4, "ohdr8")
            nc.vector.tensor_copy(ohdr8, ohdr_f)
            zcol = ptile([128, 1], dt.float32, "zcol")
            nc.vector.memset(zcol, 0.0)

            def col(src_1d, nm):
                t = ptile([C, 1], dt.float32, nm)
                nc.scalar.dma_start(out=t, in_=src_1d[:, None])
                return t

            ipb_col = ptile([C, 1], dt.float32, "ipbcol")
            nc.scalar.dma_start(out=ipb_col, in_=bass.AP(
                tensor=ipb_row.tensor, offset=ipb_row.offset, ap=[[1, C], [C, 1]]))
            conv_b_col = [col(conv_b[l], f"cb{l}") for l in range(L)]
            g_col = [col(ln_g[l], f"g{l}") for l in range(L)]
            b_col = [col(ln_b[l], f"b{l}") for l in range(L)]
            og_col = col(out_ln_g, "og")
            ob_col = col(out_ln_b, "ob")

            outbB = ptile([128, DOUT], dt.float32, "outbB")
            nc.scalar.dma_start(out=outbB, in_=bass.AP(
                tensor=out_b.tensor, offset=out_b.offset, ap=[[0, 128], [1, DOUT]]))

            inw_f = ptile([128, 2, C], dt.float32, "inwf")
            nc.scalar.dma_start(out=inw_f, in_=in_wT.rearrange("(a p) c -> p a c", p=128))
            inw_bf = ptile([128, 2, C], dt.bfloat16, "inwbf")
            nc.vector.tensor_copy(inw_bf, inw_f)
            posw_f = ptile([C, C], dt.float32, "poswf")
            nc.scalar.dma_start(out=posw_f, in_=pos_wT)
            posw_bf = ptile([C, C], dt.bfloat16, "poswbf")
            nc.vector.tensor_copy(posw_bf, posw_f)
            outw_f = ptile([C, DOUT], dt.float32, "outwf")
            nc.scalar.dma_start(out=outw_f, in_=out_wT)
            outw_bf = ptile([C, DOUT], dt.bfloat16, "outwbf")
            cw_bf = [ptile([128, K, C], dt.bfloat16, f"cw{l}")
                     for l in range(L)]

            def emit_deferred_weight_casts():
                # conv-weight staging + bf16 conversions queued only after
                # the upsample chain's ops, so the startup critical path
                # (DVE in-order queue) is not delayed
                nc.vector.tensor_copy(outw_bf, outw_f)
                for l in range(L):
                    cwf = ptile([128, K, C], dt.float32, "cwstage", pool=pp,
                                bufs=1)
                    (nc.scalar, nc.sync, nc.gpsimd)[l].dma_start(
                        out=cwf, in_=conv_wr[l].rearrange("k p c -> p k c"))
                    nc.vector.tensor_copy(cw_bf[l], cwf)

            # ---------- per-item persistent ----------
            xs, hs = [], []
            for b in range(BPC):
                xs.append(ptile([C, TP], dt.float32, f"x{b}"))
                h = ptile([C, HPAD + TP], dt.bfloat16, f"h{b}")
                nc.vector.memset(h[:, 0:HPAD], 0.0)
                hs.append(h)
            ybfs = [ptile([C, TP], dt.bfloat16, f"ybf{b}") for b in range(BPC)]

            # ---------- cum band edges (host-computed cumd) ----------
            cumws, cumw0s = [], []
            def emit_cum(b):
                cumw = ptile([128, NCH], dt.float32, f"cumw{b}", pool=pp)
                nc.sync.dma_start(out=cumw, in_=bass.AP(
                    tensor=cumd.tensor, offset=cumd.offset + b * (N + 1) + 1,
                    ap=[[1, 128], [128, NCH]]))
                cumw0 = ptile([128, NCH], dt.float32, f"cumw0{b}", pool=pp)
                nc.sync.dma_start(out=cumw0, in_=bass.AP(
                    tensor=cumd.tensor, offset=cumd.offset + b * (N + 1),
                    ap=[[1, 128], [128, NCH]]))
                cumws.append(cumw)
                cumw0s.append(cumw0)

            # ---------- per-chunk band masks ----------
            # D[m, t] = [t >= cum[m-1]] - [t >= cum[m]] over the chunk's
            # band; all masks depend only on iotG + cum so they pipeline
            # far ahead of the PE.
            Dg = {}
            def ensure_mask(b, pi):
                # lazily build D = [t>=cum0]*[t<cum1] (2 DVE ops) so mask
                # work interleaves with the consuming slab instead of
                # front-loading the vector queue
                if (b, pi) in Dg:
                    return Dg[(b, pi)]
                g, lo, hi = chunk_bands[b][pi]
                w = hi - lo
                bv0p = pp.tile([128, 1], dt.float32, tag="bvp", name="bvp",
                               bufs=4)
                nc.vector.tensor_scalar(bv0p, cumw0s[b][:, g:g + 1], float(lo),
                                        None, Alu.subtract)
                bv1p = pp.tile([128, 1], dt.float32, tag="bvp", name="bvp",
                               bufs=4)
                nc.vector.tensor_scalar(bv1p, cumws[b][:, g:g + 1], float(lo),
                                        None, Alu.subtract)
                S0 = pp.tile([128, WG], dt.bfloat16, tag="S0", name="S0", bufs=2)
                nc.vector.tensor_scalar(S0[:, 0:w], iot512[:, 0:w],
                                        bv0p, None, Alu.is_ge)
                D = pp.tile([128, WG], dt.bfloat16, tag="Dg", name="Dg",
                            bufs=NPIECE)
                nc.vector.scalar_tensor_tensor(D[:, 0:w], iot512[:, 0:w],
                                               bv1p, S0[:, 0:w],
                                               Alu.is_lt, Alu.mult)
                Dg[(b, pi)] = D
                return D

            # ---------- P'' projection (SBUF-resident, bf16) ----------
            psts = []
            def emit_proj(b):
                ptf = pp.tile([128, 2, N], dt.float32, tag="ptf", name="ptf", bufs=1)
                src = pooledT[b]
                ptb = ptile([128, 2, N], dt.bfloat16, f"ptb{b}", pool=pp, bufs=1)
                for qi, qe in enumerate((nc.sync, nc.scalar, nc.gpsimd, nc.sync)):
                    qe.dma_start(out=ptf[:, :, qi * 256:(qi + 1) * 256],
                                 in_=src[:, :, qi * 256:(qi + 1) * 256])
                    nc.vector.tensor_copy(ptb[:, :, qi * 256:(qi + 1) * 256],
                                          ptf[:, :, qi * 256:(qi + 1) * 256])
                pst = ptile([128, NCH, C], dt.bfloat16, f"pst{b}")
                for nchunk in range(NCH):
                    ppsf = psum.tile([128, F], dt.float32, tag="xps", name="xps", bufs=2)
                    pps = ppsf[:, 0:C]
                    for dh in range(2):
                        nc.tensor.matmul(pps, ptb[:, dh, 128 * nchunk:128 * (nchunk + 1)],
                                         inw_bf[:, dh, :], start=(dh == 0), stop=(dh == 1))
                    nc.vector.tensor_copy(pst[:, nchunk, :], pps)
                psts.append(pst)

            # ---------- upsample + posemb, one PSUM accumulation per slab ----
            def emit_upsample_slab(b, si):
                x = xs[b]
                sl = si * F
                w_val = max(0, min(F, T - sl))
                relb = pp.tile([128, F], dt.float32, tag="relb", name="relb", bufs=2)
                if w_val < F:
                    nc.vector.memset(relb, 0.0)
                if w_val > 0:
                    nc.gpsimd.dma_start(out=relb[:, 0:w_val], in_=bass.AP(
                        tensor=rel_pos.tensor,
                        offset=rel_pos.offset + b * T + sl,
                        ap=[[0, 128], [1, w_val]]))
                u = pp.tile([128, F], dt.float32, tag="u", name="u", bufs=2)
                nc.vector.tensor_scalar(u, relb, fc2pi, coff, Alu.mult, Alu.add)
                kf = pp.tile([128, F], dt.float32, tag="kf", name="kf", bufs=2)
                nc.vector.tensor_scalar(kf, u, 8388608.0, 8388608.0,
                                        Alu.add, Alu.subtract)
                nc.vector.tensor_tensor(u, u, kf, Alu.subtract)
                emb = pp.tile([128, F], dt.bfloat16, tag="emb", name="emb", bufs=2)
                nc.scalar.activation(emb, u, ActF.Sin, bias=zcol, scale=TWO_PI)
                for (pi, a0, a1) in slab_bands[b][si]:
                    ensure_mask(b, pi)
                pst = psts[b]
                xps = psum.tile([C, F], dt.float32, tag="xps", name="xps", bufs=2)
                nb = len(slab_bands[b][si])
                nc.tensor.matmul(xps, posw_bf, emb, start=True, stop=(nb == 0))
                for j, (pi, a0, a1) in enumerate(slab_bands[b][si]):
                    g, lo, _ = chunk_bands[b][pi]
                    nc.tensor.matmul(xps[:, a0 - sl:a1 - sl], pst[:, g, :],
                                     Dg[(b, pi)][:, a0 - lo:a1 - lo],
                                     start=False, stop=(j == nb - 1))
                nc.scalar.activation(x[:, sl:sl + F], xps, ActF.Identity,
                                     bias=ipb_col)

            # ---------- layernorm, slab-pipelined ----------
            epscol = ptile([128, 1], dt.float32, "epscol")
            nc.vector.memset(epscol, EPS)

            stats_bufs = {}

            def emit_stats_slab(b, S1, S2, si):
                # write fp8 copies of x and x^2 into the pair buffer half;
                # on the odd slab, one DoubleRow matmul per stat reduces
                # both slabs at once (contraction 256). Slab halves 0-3 and
                # 4-7 form separate PSUM groups on rows 0-3 of the same
                # tiles so row math never needs a nonzero base partition.
                x = xs[b]
                sl = si * F
                p = si // 2
                q = p % 2
                if si % 2 == 0:
                    stats_bufs[b] = (
                        pp.tile([128, 2, F], dt.float8e4, tag="xbf",
                                name="xbf", bufs=2),
                        pp.tile([128, 2, F], dt.float8e4, tag="xsq",
                                name="xsq", bufs=2))
                xp8, xq8 = stats_bufs[b]
                nc.scalar.activation(xp8[:, si % 2, :], x[:, sl:sl + F],
                                     ActF.Copy)
                nc.vector.tensor_tensor(xq8[:, si % 2, :], x[:, sl:sl + F],
                                        x[:, sl:sl + F], Alu.mult)
                if si % 2 == 1:
                    nc.tensor.matmul(S1, ohdr8[:, q, :, :], xp8,
                                     start=(q == 0), stop=(q == 1),
                                     perf_mode=mybir.MatmulPerfMode.DoubleRow)
                    nc.tensor.matmul(S2, ohdr8[:, q, :, :], xq8,
                                     start=(q == 0), stop=(q == 1),
                                     perf_mode=mybir.MatmulPerfMode.DoubleRow)

            def emit_stats_and_rowmath(key, b, si, strc):
                emit_stats_slab(b, *stats_tiles[key], si)
                if si % 4 == 3:
                    emit_rowmath_rows(*stats_tiles[key], 4, strc,
                                      4 * (si // 4))

            def emit_rowmath_rows(S1, S2, nr, strc, strc_r0):
                # LN row math on stats rows [0, nr); writes packed
                # (rstd, mu*rstd) rows [strc_r0, strc_r0+nr) of the DRAM
                # bounce strc[si, 0:2, F].
                s = slice(0, nr)
                mu = pp.tile([8, F], dt.float32, tag="mu", name="mu", bufs=1)
                nc.vector.tensor_scalar(mu[s], S1[s], 1.0 / C, None, Alu.mult)
                m2 = pp.tile([8, F], dt.float32, tag="mm2", name="mm2", bufs=1)
                nc.vector.scalar_tensor_tensor(m2[s], S1[s], 1.0 / C, mu[s],
                                               Alu.mult, Alu.mult)
                vr = pp.tile([8, F], dt.float32, tag="vr", name="vr", bufs=1)
                nc.vector.scalar_tensor_tensor(vr[s], S2[s], 1.0 / C, m2[s],
                                               Alu.mult, Alu.subtract)
                sq = pp.tile([8, F], dt.float32, tag="sq", name="sq", bufs=1)
                nc.scalar.activation(sq[s], vr[s], ActF.Sqrt, bias=epscol[s])
                rF = pp.tile([8, F], dt.float32, tag="rF", name="rF", bufs=1)
                nc.vector.reciprocal_approx_fast(rF[s], sq[s])
                rFb = pp.tile([8, F], dt.bfloat16, tag="rFb", name="rFb", bufs=1)
                nc.vector.tensor_copy(rFb[s], rF[s])
                cF = pp.tile([8, F], dt.bfloat16, tag="cF", name="cF", bufs=1)
                nc.vector.tensor_tensor(cF[s], mu[s], rF[s], Alu.mult)
                so = slice(strc_r0, strc_r0 + nr)
                nc.scalar.dma_start(out=strc[so, 0, :], in_=rFb[s])
                nc.scalar.dma_start(out=strc[so, 1, :], in_=cF[s])

            def new_strc():
                return dram.tile([8, 2, F], dt.bfloat16, tag="strc", name="strc",
                                 bufs=2)

            def emit_norm_slab(b, si, strc, gcol, bcol, out_act, out_tile, out_off):
                x = xs[b]
                sl = si * F
                bc = pp.tile([128, 2 * F], dt.bfloat16, tag="ABs", name="ABs", bufs=4)
                nc.gpsimd.dma_start(out=bc, in_=bass.AP(
                    tensor=strc.tensor, offset=strc.offset + si * 2 * F,
                    ap=[[0, 128], [1, 2 * F]]))
                w = min(F, T - sl)
                t1 = pp.tile([128, F], dt.float32, tag="t1", name="t1", bufs=3)
                nc.vector.tensor_tensor(t1[:, 0:w], x[:, sl:sl + w],
                                        bc[:, 0:w], Alu.mult)
                nc.vector.tensor_tensor(t1[:, 0:w], t1[:, 0:w],
                                        bc[:, F:F + w], Alu.subtract)
                nc.scalar.activation(out_tile[:, out_off + sl:out_off + sl + w],
                                     t1[:, 0:w], out_act, bias=bcol, scale=gcol)

            def emit_conv_slab(b, l, si):
                x, h = xs[b], hs[b]
                dil = dils[l]
                sl = si * F
                w = min(F, T - sl)
                cv = psum.tile([128, F], dt.float32, tag="cv", name="cv", bufs=2)
                for k in range(K):
                    off = HPAD + sl - (K - 1 - k) * dil
                    nc.tensor.matmul(cv[:, 0:w], cw_bf[l][:, k, :],
                                     h[:, off:off + w],
                                     start=(k == 0), stop=(k == K - 1))
                nc.vector.scalar_tensor_tensor(
                    x[:, sl:sl + w], cv[:, 0:w], conv_b_col[l], x[:, sl:sl + w],
                    Alu.add, Alu.add)

            def emit_out_chunk(b, tchunk):
                ybf = ybfs[b]
                t0 = 128 * tchunk
                nrows = min(128, T - t0)
                if nrows <= 0:
                    return
                pof = psum.tile([128, F], dt.float32, tag="cv", name="cv", bufs=2)
                po = pof[:, 0:DOUT]
                nc.tensor.matmul(po, ybf[:, t0:t0 + 128], outw_bf,
                                 start=True, stop=True)
                ost = pp.tile([128, DOUT], dt.bfloat16, tag="ost", name="ost", bufs=6)
                nc.vector.tensor_tensor(ost, po, outbB, Alu.add)
                qe = (nc.sync, nc.gpsimd, nc.scalar)[tchunk % 3]
                qe.dma_start(out=out[b, t0:t0 + nrows, :], in_=ost[:nrows, :])

            for b in range(BPC):
                emit_cum(b)
            emit_proj(0)

            owner_slab = [min(7, ((tc_ + 1) * 128 - 1) // F) for tc_ in range(NT)]
            stats_tiles = {}
            rcs = {}

            def new_stats(key):
                stats_tiles[key] = (
                    psum.tile([16, F], dt.float32, tag="st", name="st", bufs=4),
                    psum.tile([16, F], dt.float32, tag="st", name="st", bufs=4))

            def norm_args(stage, b):
                if stage < L:
                    return (g_col[stage], b_col[stage], _GELU, hs[b], HPAD)
                return (og_col, ob_col, ActF.Identity, ybfs[b], 0)

            # ---- upsample item 0 + its stats; conv starts right after ----
            new_stats(0)
            rcs[0] = new_strc()
            rcs[1] = new_strc()
            for si in range(8):
                emit_upsample_slab(0, si)
                emit_stats_and_rowmath(0, 0, si, rcs[0])
                if si == 0:
                    emit_deferred_weight_casts()
                    emit_proj(1)

            STEPS = [(0, 0), (1, 0), (0, 1), (1, 1), (0, 2), (1, 2)]
            for step, (b, stage) in enumerate(STEPS):
                na = norm_args(stage, b)
                nxt = STEPS[step + 1] if step + 1 < len(STEPS) else None
                first = step == 0
                last = step == len(STEPS) - 1
                if last:
                    rco = new_strc()      # strc for (0, out)
                    rco1 = new_strc()     # strc for (1, out)
                    new_stats("o1")
                if first:
                    new_stats(1)
                for si in range(8):
                    emit_norm_slab(b, si, rcs[b], *na)
                    emit_conv_slab(b, stage, si)
                    if first:
                        # inject item 1's upsample+stats under item 0's convs
                        emit_upsample_slab(1, si)
                        emit_stats_and_rowmath(1, 1, si, rcs[1])
                    elif not last:
                        if si == 2:
                            rcs[nxt[0]] = new_strc()
                            new_stats(nxt[0])
                        if 2 <= si <= 5:
                            for sj in ((si - 2) * 2, (si - 2) * 2 + 1):
                                emit_stats_and_rowmath(nxt[0], nxt[0], sj,
                                                       rcs[nxt[0]])
                    else:
                        # tail: (0, out) stats under (1, 2)'s convs; item 1's
                        # out-LN stats as each of its conv slabs finishes
                        if si == 2:
                            new_stats("o0")
                        if 2 <= si <= 5:
                            for sj in ((si - 2) * 2, (si - 2) * 2 + 1):
                                emit_stats_and_rowmath("o0", 0, sj, rco)
                        emit_stats_and_rowmath("o1", 1, si, rco1)
                        if si == 7:
                            nao = norm_args(L, 0)
                            oc = 0
                            for sj in range(8):
                                emit_norm_slab(0, sj, rco, *nao)
                                while oc < NT and owner_slab[oc] <= sj:
                                    emit_out_chunk(0, oc)
                                    oc += 1

            # ---- (1, out) ----
            nao = norm_args(L, 1)
            oc = 0
            for si in range(8):
                emit_norm_slab(1, si, rco1, *nao)
                while oc < NT and owner_slab[oc] <= si:
                    emit_out_chunk(1, oc)
                    oc += 1

    nc.compile()
    return nc


_NC_CACHE = {}


def _compute_bands(durations, T):
    """Union-over-cores chunk band pieces (width <= 512) and per-slab
    intersections referencing piece indices."""
    TP = _ceil_to(T, 128)
    F = TP // 8
    NCH = N // 128
    cum = np.cumsum(np.asarray(durations, np.int64), axis=1)
    cumd = np.concatenate([np.zeros((B, 1), np.int64), cum], axis=1)
    chunk_bands, slab_bands = [], []
    for b in range(BPC):
        items = list(range(b, B, BPC))
        pieces = []
        for g in range(NCH):
            lo = int(cumd[items, g * 128].min())
            hi = int(cumd[items, (g + 1) * 128].max())
            while hi > lo:
                mid = min(lo + 512, hi)
                pieces.append((g, lo, mid))
                lo = mid
        slabs = []
        for si in range(8):
            sl = si * F
            ent = []
            for pi, (g, lo, hi) in enumerate(pieces):
                a0 = max(lo, sl)
                a1 = min(hi, sl + F)
                if a1 > a0:
                    ent.append((pi, a0, a1))
            slabs.append(tuple(ent))
        chunk_bands.append(tuple(pieces))
        slab_bands.append(tuple(slabs))
    return tuple(chunk_bands), tuple(slab_bands)


def _get_nc(T, chunk_bands, slab_bands):
    key = (T, chunk_bands, slab_bands)
    if key not in _NC_CACHE:
        _NC_CACHE[key] = build_nc(T, chunk_bands, slab_bands)
    return _NC_CACHE[key]


def make_in_maps(pooled, rel_pos, in_w, in_b, pos_w, pos_b, conv_w, conv_b,
                 ln_g, ln_b, out_ln_g, out_ln_b, out_w, out_b, durations):
    shared = {
        "in_wT": np.ascontiguousarray(np.asarray(in_w, np.float32).T),
        "pos_wT": np.ascontiguousarray(np.asarray(pos_w, np.float32).T),
        "conv_wr": np.ascontiguousarray(np.asarray(conv_w, np.float32).transpose(0, 3, 2, 1)),
        "conv_b": np.asarray(conv_b, np.float32),
        "ln_g": np.asarray(ln_g, np.float32),
        "ln_b": np.asarray(ln_b, np.float32),
        "out_ln_g": np.asarray(out_ln_g, np.float32),
        "out_ln_b": np.asarray(out_ln_b, np.float32),
        "out_wT": np.ascontiguousarray(np.asarray(out_w, np.float32).T),
        "out_b": np.asarray(out_b, np.float32),
        "ipb_row": (np.asarray(in_b, np.float32)
                    + np.asarray(pos_b, np.float32))[None, :],
    }
    dur = np.asarray(durations, np.int64)
    cum = np.cumsum(dur, axis=1)
    cumd_all = np.concatenate([np.zeros((B, 1), np.int64), cum],
                              axis=1).astype(np.float32)
    pooledT_all = np.ascontiguousarray(
        np.asarray(pooled, np.float32).transpose(0, 2, 1)
        .reshape(B, 2, 128, N).transpose(0, 2, 1, 3))
    in_maps = []
    for c in range(NCORES):
        s = slice(c * BPC, (c + 1) * BPC)
        m = dict(shared)
        m["pooledT"] = np.ascontiguousarray(pooledT_all[s])
        m["cumd"] = np.ascontiguousarray(cumd_all[s])
        m["rel_pos"] = np.ascontiguousarray(np.asarray(rel_pos, np.float32)[s])
        in_maps.append(m)
    return in_maps


def kernel(**inputs):
    T = inputs["rel_pos"].shape[1]
    chunk_bands, slab_bands = _compute_bands(inputs["durations"], T)
    nc = _get_nc(T, chunk_bands, slab_bands)
    in_maps = make_in_maps(**inputs)
    res = bass_utils.run_bass_kernel_spmd(nc, in_maps, core_ids=list(range(NCORES)))
    return np.concatenate([np.asarray(res.results[c]["out"])
                           for c in range(NCORES)], axis=0).astype(np.float32)
